# revision 1
# baseline (speedup 1.0000x reference)
"""Trainium2 Bass kernel: aspect-level sentiment classification head.

  aspect[b] = mean(last_hidden_state[b, start_b:end_b, :])   (ragged spans)
  out = concat([pooled, aspect], -1) @ W.T + b

Strategy: data-parallel over batch (8 samples per core, 8 cores).  The key
observation is that only the span rows of last_hidden_state are ever needed,
so each core *gathers* just those rows from DRAM with an indirect DMA whose
row indices are computed on-device from position_indices.  Spans are padded
to L = 32*m rows (m = power of two chosen from the max span length at call
time); rows past the span end are masked to zero.  The per-sample 1/len is
folded into the mask so a single PE matmul per 128-column chunk produces the
*transposed* aspect features directly, which then feed an accumulated
12-chunk GEMM against host-pre-transposed W.
"""

import os
import sys

if "/opt/trn_rl_repo" not in sys.path:
    sys.path.insert(0, "/opt/trn_rl_repo")

import numpy as np

import concourse.bass as bass
import concourse.tile as tile
from concourse import bacc, mybir
from concourse.bass import IndirectOffsetOnAxis
from concourse.bass_utils import run_bass_kernel_spmd

F32 = mybir.dt.float32
I32 = mybir.dt.int32

B, S, H, C = 64, 4096, 768, 3
NCORES = 8
BL = B // NCORES          # samples per core
P = 128
HC = H // P               # 6 hidden chunks of 128
KC = 2 * H // P           # 12 contraction chunks in the final GEMM


def _log2(x: int) -> int:
    l = x.bit_length() - 1
    assert 1 << l == x
    return l


def build(m: int):
    """Build + compile the per-core SPMD program for spans up to 32*m rows."""
    assert m & (m - 1) == 0 and 1 <= m <= S // 32
    nblk = BL * m            # 32-row blocks per core
    G = nblk // 4            # gather groups of 128 rows
    cols = max(1, 4 // m)    # samples covered by one group
    gps = max(1, m // 4)     # groups per sample
    lm = _log2(m)

    nc = bacc.Bacc("TRN2", target_bir_lowering=False, debug=False,
                   num_devices=NCORES)
    lhs = nc.dram_tensor("lhs", [BL * S, H], F32, kind="ExternalInput").ap()
    # packed params: pooled_r at cols 0-127, w_r at 128-255, bias row at
    # 256-258 -- one DMA instead of three
    blob = nc.dram_tensor("blob", [HC * BL, 2 * P + C], F32,
                          kind="ExternalInput").ap()
    pos = nc.dram_tensor("pos", [BL, 2], I32, kind="ExternalInput").ap()
    out = nc.dram_tensor("out", [BL, C], F32, kind="ExternalOutput").ap()

    with tile.TileContext(nc) as tc:
        packed = m <= 4  # one PSUM bank for all 6 aspect chunks vs 6 banks
        with (
            tc.tile_pool(name="const", bufs=1) as cp,
            tc.tile_pool(name="work", bufs=4) as wp,
            tc.tile_pool(name="rows", bufs=4) as rp,
            tc.tile_pool(name="pmisc", bufs=1, space="PSUM") as pm,
            tc.tile_pool(name="pbc", bufs=2 if packed else 1,
                         space="PSUM") as pb,
            tc.tile_pool(name="pasp", bufs=1, space="PSUM") as pa,
        ):
            # ---- constants / params -------------------------------------
            id48 = cp.tile([HC * BL, HC * BL], F32, tag="id48")
            from concourse.masks import make_identity
            make_identity(nc, id48[:])

            pos_i = cp.tile([BL, 2], I32, tag="pos_i")
            nc.sync.dma_start(pos_i[:], pos[:, :], single_packet=True)
            pos_f = cp.tile([BL, 2], F32, tag="pos_f")
            nc.vector.tensor_copy(pos_f[:], pos_i[:])

            blob_sb = cp.tile([HC * BL, 2 * P + C], F32, tag="blob_sb")
            nc.sync.dma_start(blob_sb[:], blob[:, :])
            bias_row = blob_sb[0:1, 2 * P:2 * P + C]
            ones18 = cp.tile([1, BL], F32, tag="ones18")
            nc.gpsimd.memset(ones18[:], 1.0)

            # transpose pooled_r -> pT [128, 48] (pT[h, c*8+b] = pooled[b, c*128+h])
            pT_ps = pm.tile([P, HC * BL], F32, tag="pmisc", name="pT_ps")
            nc.tensor.transpose(pT_ps[:], blob_sb[:, 0:P], id48[:])
            pT = cp.tile([P, HC * BL], F32, tag="pT")
            nc.vector.tensor_copy(pT[:], pT_ps[:])

            # transpose w_r -> wT [128, 36] (wT[h, c*3+j] = W[j, c*128+h])
            wT_ps = pm.tile([P, KC * C], F32, tag="pmisc", name="wT_ps")
            nc.tensor.transpose(wT_ps[:], blob_sb[0:KC * C, P:2 * P],
                                id48[: KC * C, : KC * C])
            wT = cp.tile([P, KC * C], F32, tag="wT")
            nc.vector.tensor_copy(wT[:], wT_ps[:])

            # per-partition index helpers (p = partition id, u = p >> 5)
            iota_p = cp.tile([P, 1], I32, tag="iota_p")
            nc.gpsimd.iota(iota_p[:], pattern=[[1, 1]], base=0,
                           channel_multiplier=1)
            u_i = cp.tile([P, 1], I32, tag="u_i")
            nc.vector.tensor_scalar(u_i[:], iota_p[:], 5, None,
                                    mybir.AluOpType.arith_shift_right)
            pm32_i = cp.tile([P, 1], I32, tag="pm32_i")
            nc.vector.tensor_scalar(pm32_i[:], iota_p[:], 31, None,
                                    mybir.AluOpType.bitwise_and)
            pm32_f = cp.tile([P, 1], F32, tag="pm32_f")
            nc.vector.tensor_copy(pm32_f[:], pm32_i[:])
            u_f = cp.tile([P, 1], F32, tag="u_f")
            nc.vector.tensor_copy(u_f[:], u_i[:])

            # psum accumulators for transposed aspect features; for m >= 8
            # accumulation groups stay open across gather groups, so each
            # hidden chunk needs its own bank
            if packed:
                aspT_all = pa.tile([P, HC * BL], F32, tag="aspT")
                aspT_ps = [aspT_all[:, c * BL:(c + 1) * BL]
                           for c in range(HC)]
            else:
                aspT_ps = [pa.tile([P, BL], F32, tag=f"aspT{c}",
                                   name=f"aspT{c}")[:] for c in range(HC)]

            # ---- gather groups ------------------------------------------
            for g in range(G):
                # broadcast (start, end) of each partition's sample via PE:
                # ind[s, p] = 1 iff s == (4g + p//32) >> lm
                ind = wp.tile([BL, P], F32, tag="ind")
                nc.gpsimd.memset(ind[:], 1.0)
                nc.gpsimd.affine_select(
                    out=ind[:], in_=ind[:], compare_op=mybir.AluOpType.is_ge,
                    fill=0.0, base=128 * g, channel_multiplier=-32 * m,
                    pattern=[[1, P]])
                # keep where p - 32m*s + 128g <= 32m-1, negated for is_ge
                nc.gpsimd.affine_select(
                    out=ind[:], in_=ind[:], compare_op=mybir.AluOpType.is_ge,
                    fill=0.0, base=(32 * m - 1) - 128 * g,
                    channel_multiplier=32 * m, pattern=[[-1, P]])
                bc_ps = pb.tile([P, 2], F32, tag="bc")
                nc.tensor.matmul(out=bc_ps[:], lhsT=ind[:], rhs=pos_f[:],
                                 start=True, stop=True)
                bc = wp.tile([P, 2], F32, tag="bcs")
                nc.vector.tensor_copy(bc[:], bc_ps[:])
                st_f = bc[:, 0:1]
                en_f = bc[:, 1:2]

                # row-within-span and sample base offset for this group
                if m == 1:
                    jrow_f = pm32_f[:]
                    s4096_f = wp.tile([P, 1], F32, tag="s4096")
                    # (u + 4g) * 4096
                    nc.vector.tensor_scalar(
                        s4096_f[:], u_f[:], float(4 * g), 4096.0,
                        mybir.AluOpType.add, mybir.AluOpType.mult)
                else:
                    k_i = wp.tile([P, 1], I32, tag="k_i")
                    nc.vector.tensor_scalar(k_i[:], u_i[:], 4 * g, None,
                                            mybir.AluOpType.add)
                    q32_i = wp.tile([P, 1], I32, tag="q32")
                    nc.vector.tensor_scalar(
                        q32_i[:], k_i[:], m - 1, 32,
                        mybir.AluOpType.bitwise_and, mybir.AluOpType.mult)
                    jr_i = wp.tile([P, 1], I32, tag="jr_i")
                    nc.vector.tensor_add(jr_i[:], q32_i[:], pm32_i[:])
                    jrow_ft = wp.tile([P, 1], F32, tag="jrow_f")
                    nc.vector.tensor_copy(jrow_ft[:], jr_i[:])
                    jrow_f = jrow_ft[:]
                    s4_i = wp.tile([P, 1], I32, tag="s4_i")
                    nc.vector.tensor_scalar(
                        s4_i[:], k_i[:], lm, 4096,
                        mybir.AluOpType.arith_shift_right,
                        mybir.AluOpType.mult)
                    s4096_f = wp.tile([P, 1], F32, tag="s4096")
                    nc.vector.tensor_copy(s4096_f[:], s4_i[:])

                # gather row index = min(start + jrow, S-1) + 4096*s
                row_f = wp.tile([P, 1], F32, tag="row_f")
                nc.vector.tensor_add(row_f[:], st_f, jrow_f)
                idx_f = wp.tile([P, 1], F32, tag="idx_f")
                nc.vector.tensor_scalar(
                    idx_f[:], row_f[:], float(S - 1), s4096_f[:, 0:1],
                    mybir.AluOpType.min, mybir.AluOpType.add)
                idx_i = wp.tile([P, 1], I32, tag="idx_i")
                nc.vector.tensor_copy(idx_i[:], idx_f[:])

                # mask = (jrow < len) / len  (len==0 -> NaN, matches 0/0 ref)
                len_f = wp.tile([P, 1], F32, tag="len_f")
                nc.vector.tensor_sub(len_f[:], en_f, st_f)
                recip = wp.tile([P, 1], F32, tag="recip")
                nc.vector.reciprocal(recip[:], len_f[:])
                inm = wp.tile([P, 1], F32, tag="inm")
                nc.vector.tensor_tensor(out=inm[:], in0=jrow_f, in1=len_f[:],
                                        op=mybir.AluOpType.is_lt)
                inm_s = wp.tile([P, 1], F32, tag="inm_s")
                nc.vector.tensor_mul(inm_s[:], inm[:], recip[:])

                if cols == 1:
                    maskg = inm_s[:]
                else:
                    mk = wp.tile([P, cols], F32, tag="mk")
                    nc.vector.tensor_copy(mk[:], inm_s[:, 0:1].to_broadcast(
                        [P, cols]))
                    nc.gpsimd.affine_select(
                        out=mk[:], in_=mk[:],
                        compare_op=mybir.AluOpType.is_ge, fill=0.0, base=0,
                        channel_multiplier=1, pattern=[[-32 * m, cols]])
                    # keep where p - 32m*j <= 32m-1, negated for is_ge
                    nc.gpsimd.affine_select(
                        out=mk[:], in_=mk[:],
                        compare_op=mybir.AluOpType.is_ge, fill=0.0,
                        base=32 * m - 1, channel_multiplier=-1,
                        pattern=[[32 * m, cols]])
                    maskg = mk[:]

                rows_t = rp.tile([P, H], F32, tag="rows")
                nc.gpsimd.indirect_dma_start(
                    out=rows_t[:], out_offset=None, in_=lhs[:, :],
                    in_offset=IndirectOffsetOnAxis(ap=idx_i[:, 0:1], axis=0))

                # aspT[h, s] += rows[:, chunk].T @ mask
                s_lo = (4 * g) // m
                first = g % gps == 0
                last = g % gps == gps - 1
                for c in range(HC):
                    nc.tensor.matmul(
                        out=aspT_ps[c][:, s_lo:s_lo + cols],
                        lhsT=rows_t[:, c * P:(c + 1) * P], rhs=maskg,
                        start=first, stop=last)

            # ---- final GEMM: out[b, j] = sum_k featT[k, b] * wT[k, j] ----
            aspT_sb = cp.tile([P, HC * BL], F32, tag="aspT_sb")
            if packed:
                nc.vector.tensor_copy(aspT_sb[:], aspT_all[:])
            else:
                for c in range(HC):
                    nc.vector.tensor_copy(aspT_sb[:, c * BL:(c + 1) * BL],
                                          aspT_ps[c])

            out_ps = pm.tile([BL, C], F32, tag="pmisc", name="out_ps")
            for c in range(KC):
                featT = (pT[:, (c * BL):(c + 1) * BL] if c < HC
                         else aspT_sb[:, (c - HC) * BL:(c - HC + 1) * BL])
                nc.tensor.matmul(out=out_ps[:], lhsT=featT,
                                 rhs=wT[:, c * C:(c + 1) * C],
                                 start=(c == 0), stop=False)
            # bias as a rank-1 accumulation: ones[1,8].T @ bias_row[1,3]
            nc.tensor.matmul(out=out_ps[:], lhsT=ones18[:], rhs=bias_row,
                             start=False, stop=True)

            out_sb = cp.tile([BL, C], F32, tag="out_sb")
            nc.vector.tensor_copy(out_sb[:], out_ps[:])
            nc.sync.dma_start(out[:, :], out_sb[:], single_packet=True)

    nc.compile()
    return nc


_CACHE: dict[int, object] = {}


def _get(m: int):
    if m not in _CACHE:
        _CACHE[m] = build(m)
    return _CACHE[m]


def kernel(last_hidden_state, pooled_output, position_indices, W, b):
    last_hidden_state = np.ascontiguousarray(last_hidden_state,
                                             dtype=np.float32)
    pooled_output = np.ascontiguousarray(pooled_output, dtype=np.float32)
    position_indices = np.ascontiguousarray(position_indices, dtype=np.int32)
    W = np.ascontiguousarray(W, dtype=np.float32)
    b = np.ascontiguousarray(b, dtype=np.float32)

    lens = position_indices[:, 1] - position_indices[:, 0]
    maxlen = max(1, int(lens.max()))
    m = 1
    while 32 * m < maxlen:
        m *= 2
    in_maps = _make_in_maps(m, last_hidden_state, pooled_output,
                            position_indices, W, b)
    if RUN_KWARGS:
        # profiling path (test.py sets trace=True)
        res = run_bass_kernel_spmd(_get(m), in_maps,
                                   core_ids=list(range(NCORES)),
                                   **RUN_KWARGS)
        global LAST_RESULT
        LAST_RESULT = res
        results = res.results
    else:
        results = _run_fast(m, in_maps)
    return np.concatenate([results[c]["out"] for c in range(NCORES)],
                          axis=0)


# Cached-jit fast path: run_bass_kernel_spmd re-jits its PJRT wrapper on
# every call (~17s), so repeated kernel() calls would pay the full XLA +
# neuronx-cc pipeline each time.  This replicates bass2jax.run_bass_via_pjrt
# (multi-core branch) once per m and reuses the compiled executable.
_RUNNER_CACHE: dict = {}


def _get_runner(m):
    if m in _RUNNER_CACHE:
        return _RUNNER_CACHE[m]
    import jax
    from jax.experimental.shard_map import shard_map
    from jax.sharding import Mesh, PartitionSpec
    from concourse import bass2jax

    nc = _get(m)
    bass2jax.install_neuronx_cc_hook()
    assert nc.dbg_addr is None, "fast path assumes debug-free program"
    partition_name = (nc.partition_id_tensor.name
                      if nc.partition_id_tensor else None)

    in_names, out_names, out_avals = [], [], []
    for alloc in nc.m.functions[0].allocations:
        if not isinstance(alloc, mybir.MemoryLocationSet):
            continue
        name = alloc.memorylocations[0].name
        if alloc.kind == "ExternalInput":
            if name != partition_name:
                in_names.append(name)
        elif alloc.kind == "ExternalOutput":
            shape = tuple(alloc.tensor_shape)
            dtype = mybir.dt.np(alloc.dtype)
            out_names.append(name)
            out_avals.append(jax.core.ShapedArray(shape, dtype))
    n_params = len(in_names)
    n_outs = len(out_avals)
    all_names = in_names + out_names
    if partition_name is not None:
        all_names = all_names + [partition_name]

    def _body(*args):
        operands = list(args)
        if partition_name is not None:
            operands.append(bass2jax.partition_id_tensor())
        outs = bass2jax._bass_exec_p.bind(
            *operands,
            out_avals=tuple(out_avals),
            in_names=tuple(all_names),
            out_names=tuple(out_names),
            lowering_input_output_aliases=(),
            sim_require_finite=True,
            sim_require_nnan=True,
            nc=nc,
        )
        return tuple(outs)

    devices = jax.devices()[:NCORES]
    mesh = Mesh(np.asarray(devices), ("core",))
    specs = (PartitionSpec("core"),) * (n_params + n_outs)
    out_specs = (PartitionSpec("core"),) * n_outs
    sharded = jax.jit(
        shard_map(_body, mesh=mesh, in_specs=specs, out_specs=out_specs,
                  check_rep=False),
        donate_argnums=tuple(range(n_params, n_params + n_outs)),
        keep_unused=True,
    )
    runner = (sharded, in_names, out_names, out_avals, n_params)
    _RUNNER_CACHE[m] = runner
    return runner


def _run_fast(m, in_maps):
    sharded, in_names, out_names, out_avals, n_params = _get_runner(m)
    concat_in = [
        np.concatenate([np.asarray(in_maps[c][k]) for c in range(NCORES)],
                       axis=0)
        for k in in_names
    ]
    concat_zeros = [
        np.zeros((NCORES * a.shape[0], *a.shape[1:]), a.dtype)
        for a in out_avals
    ]
    out_arrs = sharded(*concat_in, *concat_zeros)
    return [
        {name: np.asarray(out_arrs[i]).reshape(NCORES, *out_avals[i].shape)[c]
         for i, name in enumerate(out_names)}
        for c in range(NCORES)
    ]


def _make_in_maps(m, last_hidden_state, pooled_output, position_indices,
                  W, b):
    w_r = W.reshape(C, KC, P).transpose(1, 0, 2).reshape(KC * C, P)
    in_maps = []
    for cid in range(NCORES):
        sl = slice(cid * BL, (cid + 1) * BL)
        bl = np.zeros((HC * BL, 2 * P + C), np.float32)
        bl[:, 0:P] = (pooled_output[sl].reshape(BL, HC, P)
                      .transpose(1, 0, 2).reshape(HC * BL, P))
        bl[0:KC * C, P:2 * P] = w_r
        bl[0, 2 * P:2 * P + C] = b
        in_maps.append({
            "lhs": last_hidden_state[sl].reshape(BL * S, H),
            "pos": position_indices[sl],
            "blob": bl,
        })
    return in_maps


# test/bench hooks (harness just calls kernel(); these stay default)
RUN_KWARGS: dict = {}
LAST_RESULT = None



# revision 3
# speedup vs baseline: 1.6440x; 1.6440x over previous
"""Trainium2 Bass kernel: aspect-level sentiment classification head.

  aspect[b] = mean(last_hidden_state[b, start_b:end_b, :])   (ragged spans)
  out = concat([pooled, aspect], -1) @ W.T + b

Strategy: data-parallel over batch, 8 cores.  All index/mask computation is
done on the HOST (it only depends on the tiny position_indices tensor), so
the device program is a straight line: load indices -> one indirect row
gather -> masked-mean matmuls -> output GEMM.  Span rows are packed TIGHTLY
(no per-sample padding) and samples are bin-packed across cores so the
per-core row count stays minimal -- for typical inputs every core needs a
single 128-row gather group.  The gather reads bf16 rows (lhs is pre-cast
on the host), halving HBM traffic, and the mean matmuls run in bf16
(1 PE pass/row instead of fp32's 4).  The small output GEMM stays fp32.
"""

import sys

if "/opt/trn_rl_repo" not in sys.path:
    sys.path.insert(0, "/opt/trn_rl_repo")

import numpy as np
import ml_dtypes

import concourse.bass as bass
import concourse.tile as tile
from concourse import bacc, mybir
from concourse.bass import IndirectOffsetOnAxis
from concourse.bass_utils import run_bass_kernel_spmd

F32 = mybir.dt.float32
BF16 = mybir.dt.bfloat16
I32 = mybir.dt.int32

B, S, H, C = 64, 4096, 768, 3
NCORES = 8
BL = B // NCORES          # samples per core
P = 128
HC = H // P               # 6 hidden chunks of 128
KC = 2 * H // P           # 12 contraction chunks in the final GEMM
MFW = HC * BL + KC * C + C  # pT 48 | wT 36 | bias 3


def build(ng: int):
    """Per-core SPMD program gathering ng*128 packed span rows."""
    nc = bacc.Bacc("TRN2", target_bir_lowering=False, debug=False,
                   num_devices=NCORES)
    lhs = nc.dram_tensor("lhs", [BL * S, H], BF16, kind="ExternalInput").ap()
    idx = nc.dram_tensor("idx", [P, ng], I32, kind="ExternalInput").ap()
    mbf = nc.dram_tensor("mbf", [P, ng * BL], BF16,
                         kind="ExternalInput").ap()
    mf = nc.dram_tensor("mf", [P, MFW], F32, kind="ExternalInput").ap()
    out = nc.dram_tensor("out", [BL, C], F32, kind="ExternalOutput").ap()

    with tile.TileContext(nc) as tc:
        with (
            tc.tile_pool(name="const", bufs=1) as cp,
            tc.tile_pool(name="rows", bufs=2) as rp,
            tc.tile_pool(name="pasp", bufs=1, space="PSUM") as pa,
            tc.tile_pool(name="pout", bufs=1, space="PSUM") as po,
        ):
            idx_sb = cp.tile([P, ng], I32, tag="idx")
            nc.sync.dma_start(idx_sb[:], idx[:, :])
            mbf_sb = cp.tile([P, ng * BL], BF16, tag="mbf")
            nc.scalar.dma_start(mbf_sb[:], mbf[:, :])
            mf_sb = cp.tile([P, MFW], F32, tag="mf")
            nc.scalar.dma_start(mf_sb[:], mf[:, :])
            pT = mf_sb[:, 0:HC * BL]
            wT = mf_sb[:, HC * BL:HC * BL + KC * C]
            bias_rep = mf_sb[0:BL, HC * BL + KC * C:MFW]

            aspT_ps = pa.tile([P, HC * BL], F32, tag="aspT")
            out_ps = po.tile([BL, C], F32, tag="out_ps")

            # pooled-part GEMM chunks only need mf -- they run while the
            # gather is in flight
            for c in range(HC):
                nc.tensor.matmul(out=out_ps[:], lhsT=pT[:, c * BL:(c + 1) * BL],
                                 rhs=wT[:, c * C:(c + 1) * C],
                                 start=(c == 0), stop=False)

            for g in range(ng):
                rows_t = rp.tile([P, H], BF16, tag="rows")
                nc.gpsimd.indirect_dma_start(
                    out=rows_t[:], out_offset=None, in_=lhs[:, :],
                    in_offset=IndirectOffsetOnAxis(ap=idx_sb[:, g:g + 1],
                                                   axis=0))
                for c in range(HC):
                    nc.tensor.matmul(
                        out=aspT_ps[:, c * BL:(c + 1) * BL],
                        lhsT=rows_t[:, c * P:(c + 1) * P],
                        rhs=mbf_sb[:, g * BL:(g + 1) * BL],
                        start=(g == 0), stop=(g == ng - 1))

            aspT_sb = cp.tile([P, HC * BL], F32, tag="aspT_sb")
            nc.vector.tensor_copy(aspT_sb[:], aspT_ps[:])
            for c in range(HC):
                nc.tensor.matmul(out=out_ps[:],
                                 lhsT=aspT_sb[:, c * BL:(c + 1) * BL],
                                 rhs=wT[:, (HC + c) * C:(HC + c + 1) * C],
                                 start=False, stop=(c == HC - 1))

            out_sb = cp.tile([BL, C], F32, tag="out_sb")
            nc.vector.tensor_add(out_sb[:], out_ps[:], bias_rep)
            nc.sync.dma_start(out[:, :], out_sb[:], single_packet=True)

    nc.compile()
    return nc


_CACHE: dict[int, object] = {}


def _get(ng: int):
    if ng not in _CACHE:
        _CACHE[ng] = build(ng)
    return _CACHE[ng]


def _plan(position_indices):
    """Bin-pack samples (exactly BL per core) to minimize max packed rows."""
    lens = (position_indices[:, 1] - position_indices[:, 0]).astype(np.int64)
    eff = np.clip(lens, 1, S)
    order = np.argsort(-eff, kind="stable")
    tot = np.zeros(NCORES, np.int64)
    cnt = np.zeros(NCORES, np.int64)
    bins = [[] for _ in range(NCORES)]
    big = np.int64(1) << 60
    for i in order:
        c = int(np.argmin(np.where(cnt < BL, tot, big)))
        bins[c].append(int(i))
        tot[c] += int(eff[i])
        cnt[c] += 1
    ng = max(1, -(-int(tot.max()) // P))
    return bins, ng


def _to_bf16(a):
    """Fast float32 -> bfloat16 with round-to-nearest-even."""
    u = np.ascontiguousarray(a, dtype=np.float32).view(np.uint32)
    rounded = (u + 0x7FFF + ((u >> 16) & 1)) >> 16
    return rounded.astype(np.uint16).view(ml_dtypes.bfloat16)


def _make_in_maps(ng, bins, last_hidden_state, pooled_output,
                  position_indices, W, b):
    lens = (position_indices[:, 1] - position_indices[:, 0]).astype(np.int64)
    starts = position_indices[:, 0].astype(np.int64)
    w_t = np.ascontiguousarray(
        W.reshape(C, KC, P).transpose(2, 1, 0).reshape(P, KC * C))
    in_maps = []
    for cid in range(NCORES):
        samples = bins[cid]
        idx = np.zeros(ng * P, np.int32)
        mval = np.zeros((ng * P, BL), np.float32)
        r = 0
        for j, s in enumerate(samples):
            ln = int(lens[s])
            st = int(starts[s])
            if ln <= 0:
                idx[r] = j * S + min(max(st, 0), S - 1)
                mval[r, j] = np.nan  # matches reference 0/0
                r += 1
            else:
                ln = min(ln, S)
                idx[r:r + ln] = j * S + np.minimum(st + np.arange(ln), S - 1)
                mval[r:r + ln, j] = 1.0 / ln
                r += ln
        mf = np.zeros((P, MFW), np.float32)
        mf[:, 0:HC * BL] = (pooled_output[samples].reshape(BL, HC, P)
                            .transpose(2, 1, 0).reshape(P, HC * BL))
        mf[:, HC * BL:HC * BL + KC * C] = w_t
        mf[0:BL, HC * BL + KC * C:MFW] = b[None, :]
        in_maps.append({
            "lhs": _to_bf16(last_hidden_state[samples].reshape(BL * S, H)),
            "idx": np.ascontiguousarray(idx.reshape(ng, P).T),
            "mbf": _to_bf16(mval.reshape(ng, P, BL).transpose(1, 0, 2)
                            .reshape(P, ng * BL)),
            "mf": mf,
        })
    return in_maps


def kernel(last_hidden_state, pooled_output, position_indices, W, b):
    last_hidden_state = np.ascontiguousarray(last_hidden_state,
                                             dtype=np.float32)
    pooled_output = np.ascontiguousarray(pooled_output, dtype=np.float32)
    position_indices = np.ascontiguousarray(position_indices, dtype=np.int32)
    W = np.ascontiguousarray(W, dtype=np.float32)
    b = np.ascontiguousarray(b, dtype=np.float32)

    bins, ng = _plan(position_indices)
    in_maps = _make_in_maps(ng, bins, last_hidden_state, pooled_output,
                            position_indices, W, b)
    if RUN_KWARGS:
        # profiling path (test.py sets trace=True)
        res = run_bass_kernel_spmd(_get(ng), in_maps,
                                   core_ids=list(range(NCORES)),
                                   **RUN_KWARGS)
        global LAST_RESULT
        LAST_RESULT = res
        results = res.results
    else:
        results = _run_fast(ng, in_maps)
    out = np.empty((B, C), np.float32)
    for cid in range(NCORES):
        out[bins[cid]] = results[cid]["out"]
    return out


# Cached-jit fast path: run_bass_kernel_spmd re-jits its PJRT wrapper on
# every call (~17s), so repeated kernel() calls would pay the full XLA +
# neuronx-cc pipeline each time.  This replicates bass2jax.run_bass_via_pjrt
# (multi-core branch) once per ng and reuses the compiled executable.
_RUNNER_CACHE: dict = {}


def _get_runner(ng):
    if ng in _RUNNER_CACHE:
        return _RUNNER_CACHE[ng]
    import jax
    from jax.experimental.shard_map import shard_map
    from jax.sharding import Mesh, PartitionSpec
    from concourse import bass2jax

    nc = _get(ng)
    bass2jax.install_neuronx_cc_hook()
    assert nc.dbg_addr is None, "fast path assumes debug-free program"
    partition_name = (nc.partition_id_tensor.name
                      if nc.partition_id_tensor else None)

    in_names, out_names, out_avals = [], [], []
    for alloc in nc.m.functions[0].allocations:
        if not isinstance(alloc, mybir.MemoryLocationSet):
            continue
        name = alloc.memorylocations[0].name
        if alloc.kind == "ExternalInput":
            if name != partition_name:
                in_names.append(name)
        elif alloc.kind == "ExternalOutput":
            shape = tuple(alloc.tensor_shape)
            dtype = mybir.dt.np(alloc.dtype)
            out_names.append(name)
            out_avals.append(jax.core.ShapedArray(shape, dtype))
    n_params = len(in_names)
    n_outs = len(out_avals)
    all_names = in_names + out_names
    if partition_name is not None:
        all_names = all_names + [partition_name]

    def _body(*args):
        operands = list(args)
        if partition_name is not None:
            operands.append(bass2jax.partition_id_tensor())
        outs = bass2jax._bass_exec_p.bind(
            *operands,
            out_avals=tuple(out_avals),
            in_names=tuple(all_names),
            out_names=tuple(out_names),
            lowering_input_output_aliases=(),
            sim_require_finite=True,
            sim_require_nnan=True,
            nc=nc,
        )
        return tuple(outs)

    devices = jax.devices()[:NCORES]
    mesh = Mesh(np.asarray(devices), ("core",))
    specs = (PartitionSpec("core"),) * (n_params + n_outs)
    out_specs = (PartitionSpec("core"),) * n_outs
    sharded = jax.jit(
        shard_map(_body, mesh=mesh, in_specs=specs, out_specs=out_specs,
                  check_rep=False),
        donate_argnums=tuple(range(n_params, n_params + n_outs)),
        keep_unused=True,
    )
    runner = (sharded, in_names, out_names, out_avals, n_params)
    _RUNNER_CACHE[ng] = runner
    return runner


def _run_fast(ng, in_maps):
    sharded, in_names, out_names, out_avals, n_params = _get_runner(ng)
    concat_in = [
        np.concatenate([np.asarray(in_maps[c][k]) for c in range(NCORES)],
                       axis=0)
        for k in in_names
    ]
    concat_zeros = [
        np.zeros((NCORES * a.shape[0], *a.shape[1:]), a.dtype)
        for a in out_avals
    ]
    out_arrs = sharded(*concat_in, *concat_zeros)
    return [
        {name: np.asarray(out_arrs[i]).reshape(NCORES, *out_avals[i].shape)[c]
         for i, name in enumerate(out_names)}
        for c in range(NCORES)
    ]


# test/bench hooks (harness just calls kernel(); these stay default)
RUN_KWARGS: dict = {}
LAST_RESULT = None
